# revision 1
# baseline (speedup 1.0000x reference)
"""Chamfer distance (CDLoss) Trainium2 Bass kernel.

Problem: B=8, N=4096, D=3.
  T[b,i,j] = ||pred[b,i] - gt[b,j]||^2
  loss = (sum_bj min_i T + sum_bi min_j T) / B

Sharding: one batch per NeuronCore (8 cores, SPMD). Each core computes
  partial_b[p] (per-partition sums of min distances for batch b)
and the host adds the 128 values per core, sums cores, divides by B.

Per-core algorithm (flash-style; the NxN matrix never leaves PSUM):
  Two symmetric passes; pass A puts pred-index i on PSUM partitions and
  gt-index j on the free axis, pass B swaps roles, so both min
  directions are free-axis reductions. Each [128,512] PSUM tile comes
  from ONE matmul with an augmented K=24 contraction that yields the
  full squared distance directly:
     T[i,j] = -2*p_i.g_j + ||g_j||^2 + ||p_i||^2
  rows 0-17: bf16 hi/mid/lo splits of the coordinates (6 cross terms
  x 3 dims; three bf16 levels carry ~24 mantissa bits -> fp32-grade
  dot products while the PE runs at full bf16 rate; fp32 matmul would
  be 4x slower); rows 18-20: ones x moving-side norm h/m/l; rows
  21-23: stationary-side norm h/m/l x ones. End-to-end relative error
  vs the fp32 reference: ~6e-6.

  The PSUM drain is the bottleneck (DVE reduce runs at 1 elem/lane/cyc
  at 0.96 GHz, a DVE op may read at most one PSUM operand, and GPSIMD
  has no PSUM port), so the drain is split between the two engines
  that can read PSUM: per 128-row tile (8 PSUM banks of distances),
  VectorE min-reduces 2 banks directly in fp32 while ScalarE copies 6
  banks to fp16 in SBUF at 1.2 GHz (distances are small positive
  values, so fp16 keeps ~2.4e-4 relative on the candidates); VectorE
  then folds the fp16 staging 3072->1536->768->384 with 2x-mode
  tensor_tensor mins and min-reduces the remainder. Engine busy per
  row-tile: DVE ~3.2us, ACT ~3.0us, PE ~1.7us (overlapped).

  Preprocessing builds the S (stationary) and R (moving) operand
  tensors [16, 4096] bf16 via per-row partition-flatten DMAs spread
  over the SP/ACT HWDGE queues plus a SWDGE queue; pass-A operands are
  emitted first so the main loop starts while pass-B rows stream in.
"""

import numpy as np

import concourse.bacc as bacc
import concourse.bass as bass
import concourse.tile as tile
from concourse import mybir
from concourse.bass_utils import run_bass_kernel_spmd

N = 4096
D = 3
B = 8
P = 128            # SBUF/PSUM partitions
KP = N // P        # 32 points per partition in the staging layout
NT = N // P        # 32 row-tiles per pass
CH = 512           # matmul moving free dim (one PSUM bank of fp32)
QF = 1024          # psum tile free size (2 banks); 4 tiles in flight
KROWS = 24         # augmented contraction rows

f32 = mybir.dt.float32
f16 = mybir.dt.float16
bf16 = mybir.dt.bfloat16

TRACE = False
LAST_RESULT = None

_nc_cache = None


def _build_bass():
    # Bacc (not raw Bass): its compile() legalizes multi-wait joins into
    # event semaphores; the TPB ISA has a single wait slot per instruction.
    nc = bacc.Bacc(
        "TRN2", target_bir_lowering=False, debug=False, num_devices=B,
        num_swdge_queues=4,
    )
    pred = nc.declare_dram_parameter("prediction", [N, D], f32, isOutput=False)
    gt = nc.declare_dram_parameter("ground_truth", [N, D], f32, isOutput=False)
    # per-partition partial sums; host adds the 128 values per core
    out_dram = nc.declare_dram_parameter("partial", [P, 1], f32, isOutput=True)

    with tile.TileContext(nc) as tc:
        with (
            tc.tile_pool(name="singles", bufs=1) as singles,
            tc.tile_pool(name="work", bufs=2) as work,
            tc.tile_pool(name="stage", bufs=3) as stage,
            tc.tile_pool(name="folds", bufs=3) as folds,
            tc.tile_pool(name="psum", bufs=4, space="PSUM") as psum,
        ):
            # ---------- preprocessing ----------
            # Build, per input tensor X:
            #   S_X [14, 4096] bf16 : stationary rows  [xh xh xl xl 1 1]
            #   R_X [14, 4096] bf16 : moving rows [-2xh -2xl -2xh -2xl nh nl]
            #   nsum [128, 1] f32   : per-partition sum of ||x||^2
            # Row pairing: sum_r S_P[r,i]*R_G[r,j] =
            #   -2*(ph+pl).(gh+gl) + (nh+nl) = -2 p.g + ||g||^2 (to ~2^-18)
            def levels(xdram, tag):
                xt = work.tile([P, KP, D], f32, tag="xt")
                nc.sync.dma_start(
                    out=xt, in_=xdram[:].rearrange("(p k) d -> p k d", p=P)
                )
                # inner [k d] -> [d k] so per-(level,dim) rows are contiguous
                # 32-element runs for the flatten DMAs below
                xr = work.tile([P, D, KP], f32, tag="xr")
                nc.vector.tensor_copy(out=xr, in_=xt[:].rearrange("p k d -> p d k"))
                # bf16 hi/mid/lo decomposition (3 levels carry ~24
                # mantissa bits -> fp32-equivalent dot products)
                def split3(val, pfx, shape):
                    h = work.tile(shape, bf16, tag=f"{pfx}h")
                    nc.vector.tensor_copy(out=h, in_=val)
                    h32 = work.tile(shape, f32, tag=f"{pfx}h32")
                    nc.vector.tensor_copy(out=h32, in_=h)
                    r1 = work.tile(shape, f32, tag=f"{pfx}r1")
                    nc.vector.tensor_sub(r1, val, h32)
                    m = work.tile(shape, bf16, tag=f"{pfx}m")
                    nc.vector.tensor_copy(out=m, in_=r1)
                    m32 = work.tile(shape, f32, tag=f"{pfx}m32")
                    nc.vector.tensor_copy(out=m32, in_=m)
                    r2 = work.tile(shape, f32, tag=f"{pfx}r2")
                    nc.vector.tensor_sub(r2, r1, m32)
                    l = work.tile(shape, bf16, tag=f"{pfx}l")
                    nc.vector.tensor_copy(out=l, in_=r2)
                    return h, m, l

                xh, xm, xl = split3(xr, "x", [P, D, KP])
                # scaled (-2) variants for the moving side
                xhm = work.tile([P, D, KP], bf16, tag="xhm")
                nc.vector.tensor_scalar_mul(xhm, xh, -2.0)
                xmm = work.tile([P, D, KP], bf16, tag="xmm")
                nc.vector.tensor_scalar_mul(xmm, xm, -2.0)
                xlm = work.tile([P, D, KP], bf16, tag="xlm")
                nc.vector.tensor_scalar_mul(xlm, xl, -2.0)
                # squared norms in fp32, then 3-level bf16 split
                sq = work.tile([P, D, KP], f32, tag="sq")
                nc.vector.tensor_mul(sq, xr, xr)
                n32 = work.tile([P, KP], f32, tag="n32")
                nc.vector.tensor_add(n32, sq[:, 0, :], sq[:, 1, :])
                nc.vector.tensor_add(n32, n32, sq[:, 2, :])
                nh, nm, nl = split3(n32, "n", [P, KP])
                return dict(xh=xh, xm=xm, xl=xl, xhm=xhm, xmm=xmm,
                            xlm=xlm, nh=nh, nm=nm, nl=nl)

            flat_engines = [nc.sync, nc.scalar, nc.gpsimd]
            flat_i = [0]

            def flat(dst, r, src2d):
                # [128, 32] staging -> one 4096-wide row (col = p*32+k),
                # round-robin across the two HWDGE queues
                eng = flat_engines[flat_i[0] % len(flat_engines)]
                flat_i[0] += 1
                eng.dma_start(
                    out=dst[r : r + 1, :].rearrange("r (p k) -> r p k", p=P),
                    in_=src2d,
                )

            ones32 = singles.tile([P, KP], bf16, tag="ones32")
            nc.vector.memset(ones32, 1.0)

            def rowcopy(dst, r0, r1, src_r0):
                # duplicate already-flattened rows (contiguous, DMA-cheap)
                eng = flat_engines[flat_i[0] % len(flat_engines)]
                flat_i[0] += 1
                eng.dma_start(
                    out=dst[r0:r1, :], in_=dst[src_r0 : src_r0 + (r1 - r0), :]
                )

            # Row pairing (S[r] * R[r] summed over r = full distance):
            #   0-2:(h,-2h) 3-5:(h,-2m) 6-8:(m,-2h) 9-11:(h,-2l)
            #   12-14:(l,-2h) 15-17:(m,-2m)  [ml/lm/ll dropped, ~2^-27]
            #   18-20:(1, n_hml)  21-23:(n_hml, 1)
            def flats_S(S, lv):
                for d in range(D):
                    flat(S, 0 + d, lv["xh"][:, d, :])
                    flat(S, 6 + d, lv["xm"][:, d, :])
                    flat(S, 12 + d, lv["xl"][:, d, :])
                flat(S, 18, ones32)
                flat(S, 19, ones32)
                flat(S, 20, ones32)
                flat(S, 21, lv["nh"])
                flat(S, 22, lv["nm"])
                flat(S, 23, lv["nl"])
                rowcopy(S, 3, 6, 0)
                rowcopy(S, 9, 12, 0)
                rowcopy(S, 15, 18, 6)

            def flats_R(R, lv):
                for d in range(D):
                    flat(R, 0 + d, lv["xhm"][:, d, :])
                    flat(R, 3 + d, lv["xmm"][:, d, :])
                    flat(R, 9 + d, lv["xlm"][:, d, :])
                flat(R, 18, lv["nh"])
                flat(R, 19, lv["nm"])
                flat(R, 20, lv["nl"])
                flat(R, 21, ones32)
                flat(R, 22, ones32)
                flat(R, 23, ones32)
                rowcopy(R, 6, 9, 0)
                rowcopy(R, 12, 15, 0)
                rowcopy(R, 15, 18, 3)

            lvP = levels(pred, "p")
            lvG = levels(gt, "g")
            S_P = singles.tile([KROWS, N], bf16, tag="S_p")
            R_P = singles.tile([KROWS, N], bf16, tag="R_p")
            S_G = singles.tile([KROWS, N], bf16, tag="S_g")
            R_G = singles.tile([KROWS, N], bf16, tag="R_g")
            # pass-A operands first so the main loop starts while the
            # pass-B flats still stream in the background
            flats_S(S_P, lvP)
            flats_R(R_G, lvG)
            flats_S(S_G, lvG)
            flats_R(R_P, lvP)

            # per-pass rowmin collectors (column it = rowmin of row-tile it)
            Md_A = singles.tile([P, NT], f32, tag="Md_A")  # fp32 direct part
            Mb_A = singles.tile([P, NT], f32, tag="Mb_A")  # fp16 staged part
            Md_B = singles.tile([P, NT], f32, tag="Md_B")
            Mb_B = singles.tile([P, NT], f32, tag="Mb_B")

            # ---------- main passes ----------
            # Per row-tile (8 PSUM banks of distances): DVE min-reduces 2
            # banks directly in fp32; ACT copies 6 banks to fp16 in SBUF
            # (distances are small positive values, so fp16 keeps ~2.4e-4
            # relative); DVE folds the staging with 2x-mode fp16 mins.
            for Md, Mb, S, R in (
                (Md_A, Mb_A, S_P, R_G),
                (Md_B, Mb_B, S_G, R_P),
            ):
                for it in range(NT):
                    lhsT = S[0:KROWS, it * P : (it + 1) * P]

                    def mm_tile(c0):
                        T = psum.tile([P, QF], f32, tag="psumT")
                        for h in range(2):
                            nc.tensor.matmul(
                                T[:, h * CH : (h + 1) * CH],
                                lhsT,
                                R[0:KROWS, (c0 + h) * CH : (c0 + h + 1) * CH],
                                start=True,
                                stop=True,
                            )
                        return T

                    t0 = mm_tile(0)
                    nc.vector.tensor_reduce(
                        out=Md[:, it : it + 1], in_=t0,
                        axis=mybir.AxisListType.X, op=mybir.AluOpType.min,
                    )
                    C = stage.tile([P, 3 * QF], f16, tag="C")
                    for q in range(3):
                        T = mm_tile(2 * (q + 1))
                        nc.scalar.copy(out=C[:, q * QF : (q + 1) * QF], in_=T)
                    # fp16 min-folds: 3072 -> 1536 -> 768 -> 384 -> [128,1]
                    F1 = folds.tile([P, 1536], f16, tag="F1")
                    nc.vector.tensor_tensor(
                        F1, C[:, 0:1536], C[:, 1536:3072], mybir.AluOpType.min
                    )
                    F2 = folds.tile([P, 768], f16, tag="F2")
                    nc.vector.tensor_tensor(
                        F2, F1[:, 0:768], F1[:, 768:1536], mybir.AluOpType.min
                    )
                    F3 = folds.tile([P, 384], f16, tag="F3")
                    nc.vector.tensor_tensor(
                        F3, F2[:, 0:384], F2[:, 384:768], mybir.AluOpType.min
                    )
                    nc.vector.tensor_reduce(
                        out=Mb[:, it : it + 1], in_=F3,
                        axis=mybir.AxisListType.X, op=mybir.AluOpType.min,
                    )

            # ---------- finals ----------
            # rowmin = min(direct, staged); partial = sum over all rowmins
            tots = []
            for Md, Mb, tag in ((Md_A, Mb_A, "A"), (Md_B, Mb_B, "B")):
                Mm = singles.tile([P, NT], f32, tag=f"Mm_{tag}")
                nc.vector.tensor_tensor(Mm, Md, Mb, mybir.AluOpType.min)
                st = singles.tile([P, 1], f32, tag=f"st_{tag}")
                nc.vector.reduce_sum(out=st, in_=Mm, axis=mybir.AxisListType.X)
                tots.append(st)
            tot = singles.tile([P, 1], f32, tag="tot")
            nc.vector.tensor_add(tot, tots[0], tots[1])
            nc.sync.dma_start(out=out_dram[:], in_=tot)

    nc.compile()
    return nc


def _get_nc():
    global _nc_cache
    if _nc_cache is None:
        _nc_cache = _build_bass()
    return _nc_cache


def kernel(prediction, ground_truth):
    global LAST_RESULT
    pred = np.ascontiguousarray(np.asarray(prediction, dtype=np.float32))
    gtr = np.ascontiguousarray(np.asarray(ground_truth, dtype=np.float32))
    assert pred.shape == (B, N, D) and gtr.shape == (B, N, D)
    nc = _get_nc()
    in_maps = [
        {"prediction": pred[b], "ground_truth": gtr[b]} for b in range(B)
    ]
    res = run_bass_kernel_spmd(nc, in_maps, list(range(B)), trace=TRACE)
    LAST_RESULT = res
    total = sum(float(np.sum(r["partial"], dtype=np.float64)) for r in res.results)
    return np.float32(total / B)

